# revision 25
# baseline (speedup 1.0000x reference)
"""Trainium2 Bass kernel for nn_DVGAE (GCN encoder + inner-product decoder).

v3 strategy (8 NeuronCores, SPMD), all per-core:
  - Edges partitioned by SOURCE core. P0: h = l2norm(x@W2.T+b2)*1.8, xw = h@Wg.T
    computed on the local 1250-node shard (bf16 PE, skinny outputs), xw kept in
    SBUF as fp8 tiles.
  - Aggregation as dense block-matmuls: host builds A[s_local, dest] fp8 blocks
    (norm weights folded, self-loops included); PE computes partial
    z1[10000,128] = A.T @ xw_local with NO gathers. Bias via K=1 matmul.
    Dest slots parity-interleaved so the partial write has 512B runs.
  - ReduceScatter(add) -> z1 local shard [1250,128] bf16 (tiny output, cheap
    collective), then ONE AllGather of (z1 fp8 | z2col bf16) 136B rows written
    strided into a 256B-row gather table.
  - z2 = l2norm(x2@W22.T)*0.8 streamed bf16 (x2 must stay bf16 for accuracy),
    skinny [*,2] matmul orientation -> near-zero PE cost.
  - Decode partitioned by source: local endpoint rows expanded on PE via
    host-built one-hot SelT fp8 matmuls (no local gather); remote endpoint via
    one dma_gather of 256B rows; wide DVE prod; bf16 add-tree reduce; sigmoids.
"""

import sys

sys.path.insert(0, "/opt/trn_rl_repo")

import numpy as np
import ml_dtypes

import concourse.bass as bass
import concourse.bacc as bacc
import concourse.mybir as mybir
import concourse.tile as tile
from concourse.bass_utils import run_bass_kernel_spmd
from concourse.masks import make_identity

P = 128
NCORES = 8
N = 10000
E = 320000
IN_DIM = 512
C = 128
NS = N // NCORES            # 1250 nodes per core
SW = 10                     # local src windows of 128 (last 98)
SPANS = 40                  # 256-node dest spans (last has 16 nodes)
KT = 80                     # z2 k-tiles of 128 (10240 padded)
KB = 4                      # k-tiles per x2 stream DMA
GC = 4                      # decode chunks per PSUM group
SCALING_FACTOR = 1.8
SC = 0.8
L2_EPS = 1e-12
PADN = N                    # remote pad index -> zeroed row
ZROWS = 10112               # z1x rows (79*128, >= N + pad)

bf16 = mybir.dt.bfloat16
fp32 = mybir.dt.float32
fp8 = mybir.dt.float8e4
i16 = mybir.dt.int16
i32 = mybir.dt.int32

_F32 = np.float32
_BF16 = ml_dtypes.bfloat16
_FP8 = ml_dtypes.float8_e4m3


def _build_program(NU):
    """NU = chunks per decode window (compile-time uniform)."""
    NQ = SW * NU             # total decode chunks
    nc = bacc.Bacc(None, target_bir_lowering=False, debug=False)

    # ---------------- I/O ----------------
    xT = nc.declare_dram_parameter("xT", [5 * P, NS], bf16, isOutput=False)
    w2T = nc.declare_dram_parameter("w2T", [5 * P, C], bf16, isOutput=False)
    wgT = nc.declare_dram_parameter("wgT", [C, C], bf16, isOutput=False)
    x2T = nc.declare_dram_parameter("x2T", [KT * P, NS], bf16, isOutput=False)
    w22T = nc.declare_dram_parameter("w22T", [KT * P, 2], bf16, isOutput=False)
    Ablk = nc.declare_dram_parameter("Ablk", [P, SPANS * 2 * SW * P], fp8, isOutput=False)
    bmask = nc.declare_dram_parameter("bmask", [1, SPANS * 2 * P], fp8, isOutput=False)
    bgrow = nc.declare_dram_parameter("bgrow", [1, C], fp32, isOutput=False)
    selT = nc.declare_dram_parameter("selT", [P, NQ * P], fp8, isOutput=False)
    ridx = nc.declare_dram_parameter("ridx", [P, NQ * 8], i16, isOutput=False)

    dec_out = nc.declare_dram_parameter("dec_out", [P, NQ], fp32, isOutput=True)

    # ------------- internal DRAM -------------
    partial = nc.dram_tensor("partial", [N, C], bf16)
    z1loc_d = nc.dram_tensor("z1loc_d", [NS, C], bf16)
    ag_in = nc.dram_tensor("ag_in", [NS, 65], bf16)
    z1_ext = nc.dram_tensor("z1_ext", [N, 65], bf16, addr_space="Shared")
    z1x = nc.dram_tensor("z1x", [ZROWS, 128], bf16)

    rg = [list(range(NCORES))]

    with tile.TileContext(nc) as tc:
        with (
            tc.tile_pool(name="const", bufs=1) as cpool,
            tc.tile_pool(name="sb", bufs=3) as sb,
            tc.tile_pool(name="x2s", bufs=2) as x2pool,
            tc.tile_pool(name="ab", bufs=3) as apool,
            tc.tile_pool(name="part", bufs=2) as ppool,
            tc.tile_pool(name="gr", bufs=2) as grpool,
            tc.tile_pool(name="selw", bufs=2) as selpool,
            tc.tile_pool(name="prod", bufs=2) as prpool,
            tc.tile_pool(name="tree", bufs=2) as trpool,
            tc.tile_pool(name="ps_h", bufs=2, space="PSUM") as psH,
            tc.tile_pool(name="ps_g", bufs=2, space="PSUM") as psG,
            tc.tile_pool(name="ps_e", bufs=2, space="PSUM") as psE,
        ):
            # ---------- constants ----------
            xT_sb = cpool.tile([P, 5, NS], bf16)
            nc.sync.dma_start(out=xT_sb[:], in_=xT[:].rearrange("(t p) n -> p t n", p=P))
            w2T_sb = cpool.tile([P, 5, C], bf16)
            nc.sync.dma_start(out=w2T_sb[:], in_=w2T[:].rearrange("(t p) c -> p t c", p=P))
            wgT_sb = cpool.tile([P, C], bf16)
            nc.sync.dma_start(out=wgT_sb[:], in_=wgT[:])
            w22T_sb = cpool.tile([P, KT, 2], bf16)
            nc.sync.dma_start(out=w22T_sb[:], in_=w22T[:].rearrange("(t p) c -> p t c", p=P))
            ridx_sb = cpool.tile([P, NQ * 8], i16)
            nc.scalar.dma_start(out=ridx_sb[:], in_=ridx[:])
            bm_sb = cpool.tile([1, SPANS * 2 * P], fp8)
            nc.scalar.dma_start(out=bm_sb[:], in_=bmask[:])
            bg_sb = cpool.tile([1, C], fp32)
            nc.scalar.dma_start(out=bg_sb[:], in_=bgrow[:])
            bgb = cpool.tile([1, C], bf16)
            nc.vector.tensor_copy(out=bgb[:], in_=bg_sb[:])

            ident = cpool.tile([P, P], fp32)
            make_identity(nc, ident[:])
            identb = cpool.tile([P, P], bf16)
            nc.vector.tensor_copy(out=identb[:], in_=ident[:])

            xw_tiles = cpool.tile([P, SW, C], fp8)
            nc.vector.memset(xw_tiles[:], 0.0)

            # zero the pad rows of the remote gather table
            zpad = cpool.tile([P, SW, 128], bf16)
            nc.vector.memset(zpad[:], 0.0)
            for zg in range(8):
                za = min(10, 79 - zg * 10)
                nc.gpsimd.dma_start(
                    out=z1x[zg * 1280 : zg * 1280 + za * P, :].rearrange(
                        "(a p) c -> p a c", p=P
                    ),
                    in_=zpad[:, :za, :],
                )

            # ---------- P0: h = l2norm(x@W2.T+b2)*1.8 ; xw = h@Wg.T (fp8) ----------
            for nb in range(SW):
                n0 = nb * P
                nw = min(P, NS - n0)
                h_ps = psH.tile([P, C], fp32, space="PSUM", tag="h")
                for t in range(5):
                    nc.tensor.matmul(
                        out=h_ps[:nw],
                        lhsT=xT_sb[:, t, n0 : n0 + nw],
                        rhs=w2T_sb[:, t, :],
                        start=(t == 0),
                        stop=(t == 4),
                    )
                sq = sb.tile([P, C], fp32, tag="sq")
                ss = sb.tile([P, 1], fp32, tag="ss")
                nc.scalar.activation(
                    out=sq[:nw], in_=h_ps[:nw],
                    func=mybir.ActivationFunctionType.Square,
                    accum_out=ss[:nw, :1],
                )
                nc.scalar.activation(
                    out=ss[:nw, :1], in_=ss[:nw, :1],
                    func=mybir.ActivationFunctionType.Sqrt,
                )
                nc.vector.tensor_scalar_max(ss[:nw, :1], ss[:nw, :1], L2_EPS)
                rinv = sb.tile([P, 1], fp32, tag="rinv")
                nc.vector.reciprocal(rinv[:nw, :1], ss[:nw, :1])
                nc.scalar.activation(
                    out=rinv[:nw, :1], in_=rinv[:nw, :1],
                    func=mybir.ActivationFunctionType.Copy, scale=SCALING_FACTOR,
                )
                h2 = sb.tile([P, C], bf16, tag="h2")
                nc.scalar.activation(
                    out=h2[:nw], in_=h_ps[:nw],
                    func=mybir.ActivationFunctionType.Copy, scale=rinv[:nw, :1],
                )
                h2T_ps = psH.tile([P, P], bf16, space="PSUM", tag="h")
                nc.tensor.matmul(
                    out=h2T_ps[:, :nw], lhsT=h2[:nw], rhs=identb[:nw, :nw],
                    is_transpose=True,
                )
                h2T = sb.tile([P, P], bf16, tag="h2T")
                nc.vector.tensor_copy(out=h2T[:, :nw], in_=h2T_ps[:, :nw])
                xw_ps = psH.tile([P, C], fp32, space="PSUM", tag="h")
                nc.tensor.matmul(
                    out=xw_ps[:nw], lhsT=h2T[:, :nw], rhs=wgT_sb[:], start=True, stop=True
                )
                nc.scalar.activation(
                    out=xw_tiles[:nw, nb, :], in_=xw_ps[:nw],
                    func=mybir.ActivationFunctionType.Copy,
                )

            # ---------- aggregation: partial z1 = A.T @ xw (dense blocks) ----------
            ABG = 4     # spans per A DMA
            for g in range(SPANS // ABG):
                at = apool.tile([P, ABG * 2 * SW * P], fp8, tag="at")
                nc.gpsimd.dma_start(
                    out=at[:],
                    in_=Ablk[:, g * ABG * 2 * SW * P : (g + 1) * ABG * 2 * SW * P],
                )
                for s4 in range(ABG):
                    sp = g * ABG + s4
                    rows = 256 if sp < SPANS - 1 else N - 256 * (SPANS - 1)
                    pp = rows // 2
                    aps = psG.tile([P, 2, C], fp32, space="PSUM", tag="agg")
                    for par in range(2):
                        w = sp * 2 + par
                        for sbk in range(SW):
                            off = ((s4 * 2 + par) * SW + sbk) * P
                            nc.tensor.matmul(
                                out=aps[:, par, :],
                                lhsT=at[:, off : off + P],
                                rhs=xw_tiles[:, sbk, :],
                                start=(sbk == 0),
                                stop=False,
                            )
                        nc.tensor.matmul(
                            out=aps[:, par, :],
                            lhsT=bm_sb[:, w * P : (w + 1) * P],
                            rhs=bgb[:],
                            start=False,
                            stop=True,
                        )
                    spart = ppool.tile([P, 2, C], bf16, tag="sp")
                    nc.scalar.activation(
                        out=spart[:], in_=aps[:],
                        func=mybir.ActivationFunctionType.Copy,
                    )
                    nc.scalar.dma_start(
                        out=partial[sp * 256 : sp * 256 + rows, :].rearrange(
                            "(p s) c -> p s c", s=2
                        ),
                        in_=spart[:pp, :, :],
                    )

            nc.gpsimd.collective_compute(
                "ReduceScatter",
                mybir.AluOpType.add,
                ins=[partial[:]],
                outs=[z1loc_d[:]],
                replica_groups=rg,
            )

            # ---------- z2 = l2norm(x2 @ W22.T) * 0.8 (skinny matmuls) ----------
            zacc0 = cpool.tile([P, 2 * SW], fp32)
            zacc1 = cpool.tile([P, 2 * SW], fp32)
            zacc = [zacc0, zacc1]
            for b in range(KT // KB):
                xt = x2pool.tile([P, KB, SW * P], bf16, tag="x2t")
                nc.vector.memset(xt[:, :, NS : SW * P], 0.0)
                nc.sync.dma_start(
                    out=xt[:, :, 0:NS],
                    in_=x2T[b * KB * P : (b + 1) * KB * P, :].rearrange(
                        "(a p) n -> p a n", p=P
                    ),
                )
                zps = psH.tile([P, 2 * SW], fp32, space="PSUM", tag="h")
                for ncb in range(SW):
                    n0 = ncb * P
                    for a in range(KB):
                        nc.tensor.matmul(
                            out=zps[:, 2 * ncb : 2 * ncb + 2],
                            lhsT=xt[:, a, n0 : n0 + P],
                            rhs=w22T_sb[:, b * KB + a, :],
                            start=(a == 0),
                            stop=(a == KB - 1),
                        )
                if b == 0:
                    nc.vector.tensor_copy(out=zacc[0][:], in_=zps[:])
                else:
                    nc.vector.tensor_tensor(
                        out=zacc[b % 2][:], in0=zacc[(b - 1) % 2][:], in1=zps[:],
                        op=mybir.AluOpType.add,
                    )
            zfin = zacc[(KT // KB - 1) % 2]

            z2colb = cpool.tile([P, SW, 1], bf16)
            nc.vector.memset(z2colb[:], 0.0)
            for ncb in range(SW):
                nw = min(P, NS - ncb * P)
                z2sq = sb.tile([P, 2], fp32, tag="z2sq")
                z2ss = sb.tile([P, 1], fp32, tag="z2ss")
                nc.scalar.activation(
                    out=z2sq[:nw], in_=zfin[:nw, 2 * ncb : 2 * ncb + 2],
                    func=mybir.ActivationFunctionType.Square,
                    accum_out=z2ss[:nw, :1],
                )
                nc.scalar.activation(
                    out=z2ss[:nw, :1], in_=z2ss[:nw, :1],
                    func=mybir.ActivationFunctionType.Sqrt,
                )
                nc.vector.tensor_scalar_max(z2ss[:nw, :1], z2ss[:nw, :1], L2_EPS)
                z2r = sb.tile([P, 1], fp32, tag="z2r")
                nc.vector.reciprocal(z2r[:nw, :1], z2ss[:nw, :1])
                nc.scalar.activation(
                    out=z2r[:nw, :1], in_=z2r[:nw, :1],
                    func=mybir.ActivationFunctionType.Copy, scale=SC,
                )
                nc.vector.tensor_scalar(
                    out=z2colb[:nw, ncb, :],
                    in0=zfin[:nw, 2 * ncb : 2 * ncb + 1],
                    scalar1=z2r[:nw, :1],
                    scalar2=None,
                    op0=mybir.AluOpType.mult,
                )

            # ---------- z1 local shard -> asm (fp8 z1 | bf16 z2col) ----------
            z1l_bf = cpool.tile([P, SW, C], bf16)
            nc.vector.memset(z1l_bf[:], 0.0)
            nc.sync.dma_start(
                out=z1l_bf[:, 0:9, :],
                in_=z1loc_d[0 : 9 * P, :].rearrange("(w p) c -> p w c", p=P),
            )
            nc.sync.dma_start(out=z1l_bf[0:98, 9, :], in_=z1loc_d[9 * P : NS, :])
            z1l_f8 = cpool.tile([P, SW, C], fp8)
            nc.vector.tensor_copy(out=z1l_f8[:], in_=z1l_bf[:])
            z1wB = cpool.tile([P, SW, C + 1], bf16)
            nc.vector.tensor_copy(out=z1wB[:, :, 0:C], in_=z1l_bf[:])
            nc.vector.tensor_copy(out=z1wB[:, :, C : C + 1], in_=z2colb[:])

            z1f8b = z1l_f8[:].bitcast(bf16)
            nc.gpsimd.dma_start(
                out=ag_in[0 : 9 * P, 0:64].rearrange("(w p) c -> p w c", p=P),
                in_=z1f8b[:, 0:9, :],
            )
            nc.gpsimd.dma_start(out=ag_in[9 * P : NS, 0:64], in_=z1f8b[0:98, 9, :])
            nc.gpsimd.dma_start(
                out=ag_in[0 : 9 * P, 64:65].rearrange("(w p) c -> p w c", p=P),
                in_=z2colb[:, 0:9, :],
            )
            nc.gpsimd.dma_start(out=ag_in[9 * P : NS, 64:65], in_=z2colb[0:98, 9, :])

            nc.gpsimd.collective_compute(
                "AllGather",
                mybir.AluOpType.bypass,
                ins=[ag_in[:]],
                outs=[z1_ext[:]],
                replica_groups=rg,
            )
            nc.scalar.dma_start(out=z1x[0:N, 0:65], in_=z1_ext[:])

            # ---------- decode ----------
            vf_all = cpool.tile([P, NQ], fp32)
            vn_all = cpool.tile([P, NQ], fp32)
            NGW = (NU + GC - 1) // GC
            for win in range(SW):
                selw = selpool.tile([P, NU * P], fp8, tag="selw")
                nc.sync.dma_start(
                    out=selw[:], in_=selT[:, win * NU * P : (win + 1) * NU * P]
                )
                grt = grpool.tile([P, NU, 64], i32, tag="gr")
                nc.gpsimd.dma_gather(
                    grt[:], z1x[:].bitcast(i32),
                    ridx_sb[:, win * NU * 8 : (win + 1) * NU * 8],
                    NU * P, NU * P, 64, elem_step=64,
                    single_packet=False,
                )
                gr8 = grt[:].bitcast(fp8)
                grb = grt[:].bitcast(bf16)
                use_bf = (win % 2 == 0)
                if use_bf:
                    grc = grpool.tile([P, NU, C], bf16, tag="grc")
                    nc.scalar.activation(
                        out=grc[:], in_=gr8[:, :, 0:128],
                        func=mybir.ActivationFunctionType.Copy,
                    )
                prodb = prpool.tile([P, NU, C], bf16, tag="pr")
                for grp in range(NGW):
                    u0 = grp * GC
                    ng = min(GC, NU - u0)
                    pE = psE.tile([P, GC, 256], fp32, space="PSUM", tag="E")
                    for u in range(u0, u0 + ng):
                        q = win * NU + u
                        nc.tensor.matmul(
                            out=pE[:, u - u0, 0:129],
                            lhsT=selw[:, (q - win * NU) * P : (q - win * NU + 1) * P],
                            rhs=z1wB[:, win, :],
                            start=True, stop=True,
                        )
                    eb = sb.tile([P, GC, 129], bf16, tag="eb")
                    nc.scalar.activation(
                        out=eb[:, 0:ng, :], in_=pE[:, 0:ng, 0:129],
                        func=mybir.ActivationFunctionType.Copy,
                    )
                    nc.vector.tensor_tensor(
                        out=prodb[:, u0 : u0 + ng, :],
                        in0=eb[:, 0:ng, 0:128],
                        in1=(grc[:, u0 : u0 + ng, :] if use_bf
                             else gr8[:, u0 : u0 + ng, 0:128]),
                        op=mybir.AluOpType.mult,
                    )
                    nc.vector.tensor_tensor(
                        out=vn_all[:, win * NU + u0 : win * NU + u0 + ng],
                        in0=eb[:, 0:ng, 128:129],
                        in1=grb[:, u0 : u0 + ng, 64:65],
                        op=mybir.AluOpType.add,
                    )
                # bf16 add-tree reduce over channels (2x DVE)
                trA = trpool.tile([P, NU, 64], bf16, tag="trA")
                trB = trpool.tile([P, NU, 32], bf16, tag="trB")
                nc.vector.tensor_tensor(
                    out=trA[:], in0=prodb[:, :, 0:64], in1=prodb[:, :, 64:128],
                    op=mybir.AluOpType.add,
                )
                nc.vector.tensor_tensor(
                    out=trB[:], in0=trA[:, :, 0:32], in1=trA[:, :, 32:64],
                    op=mybir.AluOpType.add,
                )
                nc.vector.tensor_tensor(
                    out=trA[:, :, 0:16], in0=trB[:, :, 0:16], in1=trB[:, :, 16:32],
                    op=mybir.AluOpType.add,
                )
                nc.vector.tensor_tensor(
                    out=trB[:, :, 0:8], in0=trA[:, :, 0:8], in1=trA[:, :, 8:16],
                    op=mybir.AluOpType.add,
                )
                nc.vector.tensor_tensor(
                    out=trA[:, :, 0:4], in0=trB[:, :, 0:4], in1=trB[:, :, 4:8],
                    op=mybir.AluOpType.add,
                )
                nc.vector.tensor_tensor(
                    out=trB[:, :, 0:2], in0=trA[:, :, 0:2], in1=trA[:, :, 2:4],
                    op=mybir.AluOpType.add,
                )
                nc.vector.tensor_tensor(
                    out=vf_all[:, win * NU : (win + 1) * NU],
                    in0=trB[:, :, 0:1], in1=trB[:, :, 1:2],
                    op=mybir.AluOpType.add,
                )

            sf = cpool.tile([P, NQ], fp32)
            nc.scalar.activation(
                out=sf[:], in_=vf_all[:], func=mybir.ActivationFunctionType.Sigmoid
            )
            sn = cpool.tile([P, NQ], fp32)
            nc.scalar.activation(
                out=sn[:], in_=vn_all[:], func=mybir.ActivationFunctionType.Sigmoid
            )
            t1 = cpool.tile([P, NQ], fp32)
            nc.vector.tensor_tensor(out=t1[:], in0=sf[:], in1=sf[:], op=mybir.AluOpType.mult)
            t2 = cpool.tile([P, NQ], fp32)
            nc.vector.tensor_tensor(out=t2[:], in0=sf[:], in1=sn[:], op=mybir.AluOpType.mult)
            t3 = cpool.tile([P, NQ], fp32)
            nc.vector.tensor_tensor(out=t3[:], in0=t1[:], in1=sn[:], op=mybir.AluOpType.add)
            res = cpool.tile([P, NQ], fp32)
            nc.vector.tensor_tensor(out=res[:], in0=t3[:], in1=t2[:], op=mybir.AluOpType.subtract)
            nc.gpsimd.dma_start(out=dec_out[:], in_=res[:])

    nc.finalize()
    return nc


def _wrap16(logical):
    """logical [n] int (n % 16 == 0) -> [128, n//16] i16 idx table."""
    n = logical.shape[0]
    st = logical.reshape(n // 16, 16).T.astype(np.int16)
    return np.tile(st, (8, 1))


def _prepare(x, x2, W2, b2, Wg, bg, W22, edge_index):
    x = np.asarray(x, dtype=_F32)
    x2 = np.asarray(x2, dtype=_F32)
    W2 = np.asarray(W2, dtype=_F32)
    b2 = np.asarray(b2, dtype=_F32)
    Wg = np.asarray(Wg, dtype=_F32)
    bg = np.asarray(bg, dtype=_F32)
    W22 = np.asarray(W22, dtype=_F32)
    row = np.asarray(edge_index[0], dtype=np.int64).astype(np.int32)
    col = np.asarray(edge_index[1], dtype=np.int64).astype(np.int32)

    deg = np.bincount(col, minlength=N).astype(np.float64) + 1.0
    dinv = (1.0 / np.sqrt(deg)).astype(_F32)

    # shared weight tensors
    w2T = np.zeros((5 * P, C), _BF16)
    w2T[:IN_DIM] = W2.T.astype(_BF16)
    w2T[IN_DIM] = b2.astype(_BF16)
    wgT = np.ascontiguousarray(Wg.T).astype(_BF16)
    w22T = np.zeros((KT * P, 2), _BF16)
    w22T[:N] = W22.T.astype(_BF16)
    bgrow = bg.reshape(1, C)

    # ---- per-core edge partition (by source) ----
    cores = []
    NU = 0
    for k in range(NCORES):
        lo = k * NS
        m = (row >= lo) & (row < lo + NS)
        es = row[m] - lo          # local src
        ed = col[m]               # global dest
        en = (dinv[row[m]] * dinv[col[m]]).astype(_F32)
        orig = np.nonzero(m)[0].astype(np.int64)
        # decode chunking by src window
        win = es // P
        order = np.argsort(win, kind="stable")
        es_s, ed_s, org_s, win_s = es[order], ed[order], orig[order], win[order]
        cnt = np.bincount(win_s, minlength=SW)
        NU = max(NU, int(np.ceil(cnt.max() / P)))
        cores.append((es, ed, en, es_s, ed_s, org_s, cnt, lo))

    NQ = SW * NU
    in_maps = []
    outpos = []
    for k in range(NCORES):
        es, ed, en, es_s, ed_s, org_s, cnt, lo = cores[k]

        # aggregation A blocks: [s, span, par, sb, dslot]
        A = np.zeros((P, SPANS, 2, SW, P), _F32)
        asrc = np.concatenate([es, np.arange(NS, dtype=np.int32)])
        adst = np.concatenate([ed, np.arange(lo, lo + NS, dtype=np.int32)])
        awt = np.concatenate([en, (dinv[lo : lo + NS] ** 2).astype(_F32)])
        sp = adst // 256
        par = adst % 2
        dslot = (adst % 256) // 2
        np.add.at(A, (asrc % P, sp, par, asrc // P, dslot), awt)
        Ahost = A.reshape(P, SPANS * 2 * SW * P).astype(_FP8)

        bm = np.zeros((1, SPANS * 2 * P), _FP8)
        dl = np.arange(lo, lo + NS, dtype=np.int32)
        bm[0, (dl // 256 * 2 + dl % 2) * P + (dl % 256) // 2] = 1.0

        # decode tables
        sel = np.zeros((P, NQ * P), _FP8)
        rlog = np.full(NQ * P, PADN, np.int32)
        oid = np.full(NQ * P, -1, np.int64)
        off = 0
        for w in range(SW):
            n_w = int(cnt[w])
            j = np.arange(n_w)
            qcol = w * NU + j // P
            lane = j % P
            flat = qcol * P + lane
            sel[es_s[off : off + n_w] - w * P, flat] = 1.0
            rlog[flat] = ed_s[off : off + n_w]
            oid[flat] = org_s[off : off + n_w]
            off += n_w
        ridx_t = np.concatenate(
            [_wrap16(rlog[w * NU * P : (w + 1) * NU * P]) for w in range(SW)], axis=1
        )
        real = oid >= 0
        outpos.append((oid[real], (np.arange(NQ * P) % P)[real],
                       (np.arange(NQ * P) // P)[real]))

        xTk = np.zeros((5 * P, NS), _BF16)
        xTk[:IN_DIM] = x[lo : lo + NS].T.astype(_BF16)
        xTk[IN_DIM] = 1.0
        x2Tk = np.zeros((KT * P, NS), _BF16)
        x2Tk[:N] = x2[lo : lo + NS].T.astype(_BF16)

        in_maps.append({
            "xT": xTk,
            "w2T": w2T,
            "wgT": wgT,
            "x2T": x2Tk,
            "w22T": w22T,
            "Ablk": Ahost,
            "bmask": bm,
            "bgrow": bgrow,
            "selT": sel,
            "ridx": ridx_t,
        })

    nc = _build_program(NU)
    return nc, in_maps, outpos


def kernel(x, x2, W2, b2, Wg, bg, W22, edge_index):
    nc, in_maps, outpos = _prepare(x, x2, W2, b2, Wg, bg, W22, edge_index)
    r = run_bass_kernel_spmd(nc, in_maps, list(range(NCORES)))
    global _last_results
    _last_results = r

    out = np.zeros(E, _F32)
    for k in range(NCORES):
        dec = r.results[k]["dec_out"]
        oid, lane, qcol = outpos[k]
        out[oid] = dec[lane, qcol]
    return out
